# revision 24
# baseline (speedup 1.0000x reference)
"""Causal self-attention (B=2, T=2048, C=768, H=12) on 8 TRN2 NeuronCores.

Sharding: core c -> batch b = c//4, head-group g = c%4 (heads 3g..3g+2).
Each core computes QKV for its 3 heads, causal attention, and a partial
c_proj (its heads' rows of W_proj). Host sums the 4 partials per batch.

Device layout is fully transposed (feature dim on partitions):
  xT [768, 2048], qkv^T tiles [128, 2048] bf16, scores S^T [k, q],
  y^T bf16, out^T bf16. Softmax over k (= partition dim of S^T) uses an
  appended ones-column on V: the PV matmul yields [y_unnorm^T; denom] in
  one accumulation group. No max-subtraction: scores are ~N(0,1), exp
  is fp32-safe in PSUM.

qkv m-tile packing (host must match), 5 tiles of 128 cols:
  m0: [V0|V1]  m1: [V2|Q2]  m2: [Q0|Q1]  m3: [K0|K1]  m4: [0|K2]
Q_h/K_h of each head sit at the same base partition (matmul requires
equal lhsT/rhs bases): h0 @ 0, h1 @ 64, h2 @ 64 (Q2 in m1, K2 in m4).

Schedule is a wavefront over the 4 q-chunks: QKV(t) then attn(t), with
QKV(t+1) / V-transposes(t+1) / proj(t-1) interleaved into the attention
stream as PE filler so the Scalar engine's exp work overlaps PE from
the start and the PE stays dense (keeps its 2.4GHz p-state).  The attn
inner loop processes k-tile PAIRS (one exp per [128,1024] pair tile)
with a one-pair lookahead, same PSUM discipline as the single shared
[128,1024]x3 pool + [128,512]x2 PV-accumulator pool (8 banks total).
Softmax normalization runs reciprocal_approx_fast on the [1,512]
denominator row BEFORE the partition broadcast (the old reciprocal on
the broadcast [64,512] tile was 3.3us per call on DVE).
"""

import numpy as np
import ml_dtypes

import concourse.bass as bass
import concourse.mybir as mybir
import concourse.tile as tile
from concourse import bacc
from concourse.bass_utils import run_bass_kernel_spmd
from concourse.masks import make_identity, make_upper_triangular

F32 = mybir.dt.float32
BF16 = mybir.dt.bfloat16
AF = mybir.ActivationFunctionType

T = 2048           # sequence length
C = 768            # embed dim
HPC = 3            # heads per core
D = 64             # head dim
NM = 5             # qkv m-tiles per core
QC = 512           # q-chunk (psum bank width in fp32)
KT = 128           # k-tile
NKT = T // KT      # 16
NQC = T // QC      # 4
NCH = C // 128     # 6 contraction chunks for qkv
SCALE = 1.0 / 8.0  # 1/sqrt(64)

_CACHE = {}
LAST_RESULTS = None
_TCNT = [0]


def mk_persist(pool, shape, dtype, name=None):
    if name is None:
        _TCNT[0] += 1
        name = f"pt{_TCNT[0]}"
    return pool.tile(shape, dtype, name=name, tag=name)


def build():
    nc = bacc.Bacc("TRN2", target_bir_lowering=False)

    xTb = nc.dram_tensor("xTb", [C, T], BF16, kind="ExternalInput")
    wqkvb = nc.dram_tensor("wqkvb", [C, NM * 128], BF16, kind="ExternalInput")
    battn = nc.dram_tensor("battn", [128, NM], F32, kind="ExternalInput")
    wproj = nc.dram_tensor("wproj", [192, C], BF16, kind="ExternalInput")
    yTb = nc.dram_tensor("yTb", [C, T], BF16, kind="ExternalOutput")

    # [768, n] dram views as [128, 6, n] so one dma_start moves all six
    # 128-row contraction chunks (40 serial descriptor builds -> ~10)
    wqkv_r = wqkvb[:, :].rearrange("(a p) n -> p a n", p=128)
    xT_r = xTb[:, :].rearrange("(a p) n -> p a n", p=128)
    yT_r = yTb[:, :].rearrange("(a p) n -> p a n", p=128)

    with tile.TileContext(nc) as tc, \
            tc.tile_pool(name="persist", bufs=1) as pp:
        # ---- persistent SBUF tensors ----
        ident_s = mk_persist(pp, [128, 128], F32)
        trimask_s = mk_persist(pp, [128, 128], F32)  # [k, q] = 1.0 iff k <= q
        make_identity(nc, ident_s[:, :])
        make_upper_triangular(nc, trimask_s[:, :], val=1.0, diag=True)
        ident = mk_persist(pp, [128, 128], BF16)
        trimask = mk_persist(pp, [128, 128], BF16)
        nc.vector.tensor_copy(ident[:, :], ident_s[:, :])
        nc.vector.tensor_copy(trimask[:, :], trimask_s[:, :])

        xsbb = mk_persist(pp, [128, NCH, T], BF16)   # x^T bf16
        wqkv_b = mk_persist(pp, [128, NCH, NM * 128], BF16)
        battn_sb = mk_persist(pp, [128, NM], F32)
        wproj_sb0 = mk_persist(pp, [128, C], BF16)   # wproj rows 0:128
        wproj_sb1 = mk_persist(pp, [64, C], BF16)    # rows 128:192
        v01 = mk_persist(pp, [128, T], BF16)   # m0: [V0|V1]
        v2q2 = mk_persist(pp, [128, T], BF16)  # m1: [V2|Q2]
        qA = mk_persist(pp, [128, T], BF16)    # m2: [Q0|Q1]
        kA = mk_persist(pp, [128, T], BF16)    # m3: [K0|K1]
        k2 = mk_persist(pp, [128, T], BF16)    # m4: [0|K2]
        vaug = mk_persist(pp, [128, NKT * HPC, 65], BF16)  # V^T tiles + ones col
        yA = mk_persist(pp, [128, T], BF16)    # y^T heads 0,1
        yB = mk_persist(pp, [64, T], BF16)     # y^T head 2
        ones64 = mk_persist(pp, [1, 64], BF16)  # keep-warm rank-1 lhsT

        # weights for the first m-tiles + x t-chunk 0 first so the first
        # QKV matmul starts as early as possible; both split by cc pairs so
        # the first accumulation group starts on chunk 0 while chunks 1-5
        # are still in flight (sub-tile deps track per-range readiness).
        for cc in range(0, NCH, 2):
            nc.sync.dma_start(wqkv_b[:, cc:cc + 2, 0:256],
                              wqkv_r[:, cc:cc + 2, 0:256])
            nc.sync.dma_start(xsbb[:, cc:cc + 2, 0:QC],
                              xT_r[:, cc:cc + 2, 0:QC])
        nc.sync.dma_start(battn_sb[:, :], battn[:, :])
        nc.sync.dma_start(wqkv_b[:, :, 256:NM * 128], wqkv_r[:, :, 256:NM * 128])
        nc.sync.dma_start(wproj_sb0[:, :], wproj[0:128, :])
        nc.sync.dma_start(wproj_sb1[:, :], wproj[128:192, :])
        for t in range(1, NQC):
            nc.sync.dma_start(xsbb[:, :, t * QC:(t + 1) * QC],
                              xT_r[:, :, t * QC:(t + 1) * QC])

        qkv_dest = [v01, v2q2, qA, kA, k2]
        # per head: (Q tile, K tile, base row)
        qk_of = [(qA, kA, 0), (qA, kA, 64), (v2q2, k2, 64)]

        with (
            tc.tile_pool(name="psA", bufs=3, space="PSUM") as psA,
            tc.tile_pool(name="psY", bufs=2, space="PSUM") as psY,
            tc.tile_pool(name="sb", bufs=8) as sbp,
            tc.tile_pool(name="sbo", bufs=2) as sbo,
        ):
            # single strided memset for every vaug ones-column
            nc.vector.memset(vaug[:, :, 64:65], 1.0)
            nc.vector.memset(ones64[:, :], 1.0)

            def emit_qkv(m, t):
                dest = qkv_dest[m]
                ps = psA.tile([128, 2 * QC], F32, tag="ps", name="ps")
                for cc in range(NCH):
                    nc.tensor.matmul(
                        ps[:, 0:QC],
                        lhsT=wqkv_b[:, cc, m * 128:(m + 1) * 128],
                        rhs=xsbb[:, cc, t * QC:(t + 1) * QC],
                        start=(cc == 0), stop=(cc == NCH - 1),
                    )
                nc.vector.tensor_scalar_add(
                    dest[:, t * QC:(t + 1) * QC], ps[:, 0:QC],
                    battn_sb[:, m:m + 1],
                )

            def emit_vtrans(t):
                # one [128,128] transpose per (v-tile, kt): cols 0:64 of the
                # v01 transpose are V0^T, 64:128 are V1^T; v2q2 gives V2^T.
                pt = psA.tile([128, 2 * QC], F32, tag="ps", name="ps") \
                    .bitcast(BF16).rearrange("p (j g c) -> p j g c", j=8, g=2, c=128)
                for j in range(4):
                    kt = 4 * t + j
                    nc.tensor.transpose(
                        pt[:, j, 0, :], v01[:, kt * KT:(kt + 1) * KT],
                        ident[:, :])
                    nc.tensor.transpose(
                        pt[:, j, 1, :], v2q2[:, kt * KT:(kt + 1) * KT],
                        ident[:, :])
                base = 4 * t
                nc.vector.tensor_copy(
                    vaug[:, 0 * NKT + base:0 * NKT + base + 4, 0:64],
                    pt[:, 0:4, 0, 0:64])
                nc.vector.tensor_copy(
                    vaug[:, 1 * NKT + base:1 * NKT + base + 4, 0:64],
                    pt[:, 0:4, 0, 64:128])
                nc.vector.tensor_copy(
                    vaug[:, 2 * NKT + base:2 * NKT + base + 4, 0:64],
                    pt[:, 0:4, 1, 0:64])

            def qlo_of_t(t, kt):
                dm = kt - 4 * t
                return 128 * dm if dm >= 0 else 0

            def emit_S_for(h, t, p, tag="pT"):
                qt, kt_t, qb = qk_of[h]
                qlo_g = t * QC
                ps = psA.tile([128, 2 * QC], F32, tag="ps", name="ps")
                pT = sbp.tile([128, 2 * QC], BF16, tag=tag, name=tag)
                for half in range(2):
                    kt = 2 * p + half
                    qlo = qlo_of_t(t, kt)
                    nc.tensor.matmul(
                        ps[:, half * QC + qlo:(half + 1) * QC],
                        lhsT=kt_t[qb:qb + 64, kt * KT:(kt + 1) * KT],
                        rhs=qt[qb:qb + 64, qlo_g + qlo:qlo_g + QC],
                        start=True, stop=True,
                    )
                lo = qlo_of_t(t, 2 * p)
                nc.scalar.activation(
                    pT[:, lo:2 * QC], ps[:, lo:2 * QC], AF.Exp,
                    scale=SCALE,
                )
                for half in range(2):
                    kt = 2 * p + half
                    if kt - 4 * t >= 0:
                        o = half * QC + qlo_of_t(t, kt)
                        nc.vector.tensor_mul(
                            pT[:, o:o + 128], pT[:, o:o + 128],
                            trimask[:, :],
                        )
                return pT

            def emit_attn(h, t, pop_filler, pre=None, last=False):
                qt, kt_t, qb = qk_of[h]
                ydest, yrow = (yA, 0) if h == 0 else (yA, 64) if h == 1 else (yB, 0)
                qlo_g = t * QC
                py = psY.tile([128, QC], F32, tag="py", name="py")
                n_k = 4 * (t + 1)
                n_pair = n_k // 2

                def qlo_of(kt):
                    return qlo_of_t(t, kt)

                def emit_S(p):
                    if pre is not None and (h, p) in pre:
                        return pre.pop((h, p))
                    return emit_S_for(h, t, p)

                def emit_PV(p, pT):
                    for half in range(2):
                        kt = 2 * p + half
                        qlo = qlo_of(kt)
                        nc.tensor.matmul(
                            py[0:65, qlo:QC],
                            lhsT=vaug[:, h * NKT + kt, :],
                            rhs=pT[:, half * QC + qlo:(half + 1) * QC],
                            start=(kt == 0), stop=(kt == n_k - 1),
                        )

                pTs = {0: emit_S(0)}
                for p in range(n_pair):
                    if p + 1 < n_pair:
                        pTs[p + 1] = emit_S(p + 1)
                    pop_filler()
                    emit_PV(p, pTs.pop(p))
                    yield

                # softmax normalize: copy the denom row off PSUM (also
                # shifts partition 64 -> 0), reciprocal, then broadcast.
                den = sbp.tile([1, QC], F32, tag="den", name="den")
                nc.vector.tensor_copy(den[0:1, :], py[64:65, :])
                rec = sbp.tile([1, QC], F32, tag="rec", name="rec")
                nc.vector.reciprocal_approx_fast(rec[0:1, :], den[0:1, :])
                bcast = sbp.tile([64, QC], F32, tag="bcast", name="bcast")
                nc.gpsimd.partition_broadcast(bcast[:, :], rec[0:1, :])
                nc.vector.tensor_mul(
                    ydest[yrow:yrow + 64, qlo_g:qlo_g + QC],
                    py[0:64, :], bcast[:, :],
                )

            def emit_proj(ct, t, osb, last):
                ps = psA.tile([128, 2 * QC], F32, tag="ps", name="ps")
                nc.tensor.matmul(
                    ps[:, 0:QC],
                    lhsT=wproj_sb0[:, ct * 128:(ct + 1) * 128],
                    rhs=yA[:, t * QC:(t + 1) * QC],
                    start=True, stop=False,
                )
                nc.tensor.matmul(
                    ps[:, 0:QC],
                    lhsT=wproj_sb1[:, ct * 128:(ct + 1) * 128],
                    rhs=yB[:, t * QC:(t + 1) * QC],
                    start=False, stop=True,
                )
                if last and ct % 2 == 0:
                    # split tail psum->sbuf copies across ACT and DVE so the
                    # copy stream keeps up with the projection matmuls
                    nc.scalar.activation(osb[:, ct, :], ps[:, 0:QC], AF.Copy)
                else:
                    nc.vector.tensor_copy(osb[:, ct, :], ps[:, 0:QC])
                if last:
                    # tail: ship each 128-row block as soon as it's ready
                    nc.sync.dma_start(
                        yT_r[:, ct, t * QC:(t + 1) * QC], osb[:, ct, :])

            # ---- schedule: wavefront over q-chunks ----
            for m in range(NM):
                emit_qkv(m, 0)
            emit_vtrans(0)

            pre_pT = {}
            for t in range(NQC):
                fillers = []
                if t + 1 < NQC:
                    for m in range(NM):
                        fillers.append(lambda m=m: emit_qkv(m, t + 1))
                    fillers.append(lambda: emit_vtrans(t + 1))
                if t - 1 >= 0:
                    osb = sbo.tile([128, NCH, QC], BF16, tag="osb", name="osb")
                    for ct in range(NCH):
                        fillers.append(
                            lambda ct=ct, osb=osb: emit_proj(ct, t - 1, osb, False))
                    fillers.append(lambda tt=t - 1, osb=osb: nc.sync.dma_start(
                        yT_r[:, :, tt * QC:(tt + 1) * QC], osb[:, :, :]))
                if t == NQC - 2:
                    # pre-compute the first two score pairs per head of the
                    # LAST chunk during this chunk (ACT has slack here); the
                    # last chunk's exp stream shrinks from 24 to 18 pairs so
                    # the Scalar engine stops gating the final drain.
                    def mk_pre(h, p):
                        def f():
                            pre_pT[(h, p)] = emit_S_for(h, NQC - 1, p, "pT3")
                        return f
                    for p in range(2):
                        for h in range(HPC):
                            fillers.append(mk_pre(h, p))

                # spread fillers evenly across this chunk's pair slots
                n_slots = HPC * 2 * (t + 1)
                fi = [0]

                def pop_filler(slot=[0], fi=fi, fillers=fillers, n_slots=n_slots):
                    slot[0] += 1
                    want = (slot[0] * len(fillers)) // n_slots
                    while fi[0] < want:
                        fillers[fi[0]]()
                        fi[0] += 1

                for h in range(HPC):
                    for _ in emit_attn(h, t, pop_filler,
                                       pre=pre_pT if t == NQC - 1 else None,
                                       last=(t == NQC - 1 and h == HPC - 1)):
                        pass
                while fi[0] < len(fillers):
                    fillers[fi[0]]()
                    fi[0] += 1

            osb = sbo.tile([128, NCH, QC], BF16, tag="osb", name="osb")
            for ct in range(NCH):
                emit_proj(ct, NQC - 1, osb, True)

    nc.finalize()
    return nc


def kernel(x, W_attn, b_attn, W_proj, b_proj):
    global LAST_RESULTS
    B = x.shape[0]
    x = np.asarray(x, np.float32)
    W_attn = np.asarray(W_attn, np.float32)
    b_attn = np.asarray(b_attn, np.float32)
    W_proj = np.asarray(W_proj, np.float32)
    b_proj = np.asarray(b_proj, np.float32)

    if "nc" not in _CACHE:
        _CACHE["nc"] = build()
    nc = _CACHE["nc"]

    in_maps = []
    for c in range(8):
        b, g = divmod(c, 4)
        heads = [3 * g + i for i in range(HPC)]
        h0, h1, h2 = heads
        Q = lambda h: W_attn[:, 64 * h:64 * h + 64]
        K = lambda h: W_attn[:, C + 64 * h:C + 64 * h + 64]
        V = lambda h: W_attn[:, 2 * C + 64 * h:2 * C + 64 * h + 64]
        bQ = lambda h: b_attn[64 * h:64 * h + 64]
        bK = lambda h: b_attn[C + 64 * h:C + 64 * h + 64]
        bV = lambda h: b_attn[2 * C + 64 * h:2 * C + 64 * h + 64]
        # m-tiles: [V0|V1], [V2|Q2], [Q0|Q1], [K0|K1], [0|K2]
        z64 = np.zeros((C, 64), np.float32)
        wqkvb = np.ascontiguousarray(np.concatenate(
            [V(h0), V(h1), V(h2), Q(h2), Q(h0), Q(h1), K(h0), K(h1),
             z64, K(h2)], 1)).astype(ml_dtypes.bfloat16)
        bcols = [bV(h0), bV(h1), bV(h2), bQ(h2), bQ(h0), bQ(h1),
                 bK(h0), bK(h1), np.zeros(64, np.float32), bK(h2)]
        bvec = np.concatenate(bcols)                     # [640] = 5 x 128
        battn = np.ascontiguousarray(bvec.reshape(NM, 128).T)  # [128, 5]
        wproj = np.concatenate(
            [W_proj[64 * h:64 * h + 64, :] for h in heads], 0)  # [192, C]
        xt = np.ascontiguousarray(x[b].T)
        in_maps.append({
            "xTb": xt.astype(ml_dtypes.bfloat16),
            "wqkvb": wqkvb,
            "battn": battn,
            "wproj": np.ascontiguousarray(wproj).astype(ml_dtypes.bfloat16),
        })

    res = run_bass_kernel_spmd(nc, in_maps, core_ids=list(range(8)))
    LAST_RESULTS = res

    out = np.zeros((B, T, C), np.float32)
    for c in range(8):
        b = c // 4
        out[b] += res.results[c]["yTb"].astype(np.float32).T
    out += b_proj
    return out


# revision 25
# speedup vs baseline: 1.0233x; 1.0233x over previous
"""Causal self-attention (B=2, T=2048, C=768, H=12) on 8 TRN2 NeuronCores.

Sharding: core c -> batch b = c//4, head-group g = c%4 (heads 3g..3g+2).
Each core computes QKV for its 3 heads, causal attention, and a partial
c_proj (its heads' rows of W_proj). Host sums the 4 partials per batch.

Device layout is fully transposed (feature dim on partitions):
  xT [768, 2048], qkv^T tiles [128, 2048] bf16, scores S^T [k, q],
  y^T bf16, out^T bf16. Softmax over k (= partition dim of S^T) uses an
  appended ones-column on V: the PV matmul yields [y_unnorm^T; denom] in
  one accumulation group. No max-subtraction: scores are ~N(0,1), exp
  is fp32-safe in PSUM.

qkv m-tile packing (host must match), 5 tiles of 128 cols:
  m0: [V0|V1]  m1: [V2|Q2]  m2: [Q0|Q1]  m3: [K0|K1]  m4: [0|K2]
Q_h/K_h of each head sit at the same base partition (matmul requires
equal lhsT/rhs bases): h0 @ 0, h1 @ 64, h2 @ 64 (Q2 in m1, K2 in m4).

Schedule is a wavefront over the 4 q-chunks: QKV(t) then attn(t), with
QKV(t+1) / V-transposes(t+1) / proj(t-1) interleaved into the attention
stream as PE filler so the Scalar engine's exp work overlaps PE from
the start and the PE stays dense (keeps its 2.4GHz p-state).  The attn
inner loop processes k-tile PAIRS (one exp per [128,1024] pair tile)
with a one-pair lookahead, same PSUM discipline as the single shared
[128,1024]x3 pool + [128,512]x2 PV-accumulator pool (8 banks total).
Softmax normalization runs reciprocal_approx_fast on the [1,512]
denominator row BEFORE the partition broadcast (the old reciprocal on
the broadcast [64,512] tile was 3.3us per call on DVE).
"""

import numpy as np
import ml_dtypes

import concourse.bass as bass
import concourse.mybir as mybir
import concourse.tile as tile
from concourse import bacc
from concourse.bass_utils import run_bass_kernel_spmd
from concourse.masks import make_identity, make_upper_triangular

F32 = mybir.dt.float32
BF16 = mybir.dt.bfloat16
AF = mybir.ActivationFunctionType

T = 2048           # sequence length
C = 768            # embed dim
HPC = 3            # heads per core
D = 64             # head dim
NM = 5             # qkv m-tiles per core
QC = 512           # q-chunk (psum bank width in fp32)
KT = 128           # k-tile
NKT = T // KT      # 16
NQC = T // QC      # 4
NCH = C // 128     # 6 contraction chunks for qkv
SCALE = 1.0 / 8.0  # 1/sqrt(64)

_CACHE = {}
LAST_RESULTS = None
_TCNT = [0]


def mk_persist(pool, shape, dtype, name=None):
    if name is None:
        _TCNT[0] += 1
        name = f"pt{_TCNT[0]}"
    return pool.tile(shape, dtype, name=name, tag=name)


def build():
    nc = bacc.Bacc("TRN2", target_bir_lowering=False)

    xTb = nc.dram_tensor("xTb", [C, T], BF16, kind="ExternalInput")
    wqkvb = nc.dram_tensor("wqkvb", [C, NM * 128], BF16, kind="ExternalInput")
    battn = nc.dram_tensor("battn", [128, NM], F32, kind="ExternalInput")
    wproj = nc.dram_tensor("wproj", [192, C], BF16, kind="ExternalInput")
    yTb = nc.dram_tensor("yTb", [C, T], BF16, kind="ExternalOutput")

    # [768, n] dram views as [128, 6, n] so one dma_start moves all six
    # 128-row contraction chunks (40 serial descriptor builds -> ~10)
    wqkv_r = wqkvb[:, :].rearrange("(a p) n -> p a n", p=128)
    xT_r = xTb[:, :].rearrange("(a p) n -> p a n", p=128)
    yT_r = yTb[:, :].rearrange("(a p) n -> p a n", p=128)

    with tile.TileContext(nc) as tc, \
            tc.tile_pool(name="persist", bufs=1) as pp:
        # ---- persistent SBUF tensors ----
        ident_s = mk_persist(pp, [128, 128], F32)
        trimask_s = mk_persist(pp, [128, 128], F32)  # [k, q] = 1.0 iff k <= q
        make_identity(nc, ident_s[:, :])
        make_upper_triangular(nc, trimask_s[:, :], val=1.0, diag=True)
        ident = mk_persist(pp, [128, 128], BF16)
        trimask = mk_persist(pp, [128, 128], BF16)
        nc.vector.tensor_copy(ident[:, :], ident_s[:, :])
        nc.vector.tensor_copy(trimask[:, :], trimask_s[:, :])

        xsbb = mk_persist(pp, [128, NCH, T], BF16)   # x^T bf16
        wqkv_b = mk_persist(pp, [128, NCH, NM * 128], BF16)
        battn_sb = mk_persist(pp, [128, NM], F32)
        wproj_sb0 = mk_persist(pp, [128, C], BF16)   # wproj rows 0:128
        wproj_sb1 = mk_persist(pp, [64, C], BF16)    # rows 128:192
        v01 = mk_persist(pp, [128, T], BF16)   # m0: [V0|V1]
        v2q2 = mk_persist(pp, [128, T], BF16)  # m1: [V2|Q2]
        qA = mk_persist(pp, [128, T], BF16)    # m2: [Q0|Q1]
        kA = mk_persist(pp, [128, T], BF16)    # m3: [K0|K1]
        k2 = mk_persist(pp, [128, T], BF16)    # m4: [0|K2]
        vaug = mk_persist(pp, [128, NKT * HPC, 65], BF16)  # V^T tiles + ones col
        yA = mk_persist(pp, [128, T], BF16)    # y^T heads 0,1
        yB = mk_persist(pp, [64, T], BF16)     # y^T head 2
        ones64 = mk_persist(pp, [1, 64], BF16)  # keep-warm rank-1 lhsT

        # weights for the first m-tiles + x t-chunk 0 first so the first
        # QKV matmul starts as early as possible; both split by cc pairs so
        # the first accumulation group starts on chunk 0 while chunks 1-5
        # are still in flight (sub-tile deps track per-range readiness).
        for cc in range(0, NCH, 2):
            nc.sync.dma_start(wqkv_b[:, cc:cc + 2, 0:256],
                              wqkv_r[:, cc:cc + 2, 0:256])
            nc.sync.dma_start(xsbb[:, cc:cc + 2, 0:QC],
                              xT_r[:, cc:cc + 2, 0:QC])
        nc.sync.dma_start(battn_sb[:, :], battn[:, :])
        nc.sync.dma_start(wqkv_b[:, :, 256:NM * 128], wqkv_r[:, :, 256:NM * 128])
        nc.sync.dma_start(wproj_sb0[:, :], wproj[0:128, :])
        nc.sync.dma_start(wproj_sb1[:, :], wproj[128:192, :])
        for t in range(1, NQC):
            nc.sync.dma_start(xsbb[:, :, t * QC:(t + 1) * QC],
                              xT_r[:, :, t * QC:(t + 1) * QC])

        qkv_dest = [v01, v2q2, qA, kA, k2]
        # per head: (Q tile, K tile, base row)
        qk_of = [(qA, kA, 0), (qA, kA, 64), (v2q2, k2, 64)]

        with (
            tc.tile_pool(name="psA", bufs=3, space="PSUM") as psA,
            tc.tile_pool(name="psY", bufs=2, space="PSUM") as psY,
            tc.tile_pool(name="sb", bufs=8) as sbp,
            tc.tile_pool(name="sbo", bufs=2) as sbo,
        ):
            # single strided memset for every vaug ones-column
            nc.vector.memset(vaug[:, :, 64:65], 1.0)
            nc.vector.memset(ones64[:, :], 1.0)

            def emit_qkv(m, t):
                dest = qkv_dest[m]
                ps = psA.tile([128, 2 * QC], F32, tag="ps", name="ps")
                for cc in range(NCH):
                    nc.tensor.matmul(
                        ps[:, 0:QC],
                        lhsT=wqkv_b[:, cc, m * 128:(m + 1) * 128],
                        rhs=xsbb[:, cc, t * QC:(t + 1) * QC],
                        start=(cc == 0), stop=(cc == NCH - 1),
                    )
                nc.vector.tensor_scalar_add(
                    dest[:, t * QC:(t + 1) * QC], ps[:, 0:QC],
                    battn_sb[:, m:m + 1],
                )

            def emit_vtrans(t):
                # one [128,128] transpose per (v-tile, kt): cols 0:64 of the
                # v01 transpose are V0^T, 64:128 are V1^T; v2q2 gives V2^T.
                pt = psA.tile([128, 2 * QC], F32, tag="ps", name="ps") \
                    .bitcast(BF16).rearrange("p (j g c) -> p j g c", j=8, g=2, c=128)
                for j in range(4):
                    kt = 4 * t + j
                    nc.tensor.transpose(
                        pt[:, j, 0, :], v01[:, kt * KT:(kt + 1) * KT],
                        ident[:, :])
                    nc.tensor.transpose(
                        pt[:, j, 1, :], v2q2[:, kt * KT:(kt + 1) * KT],
                        ident[:, :])
                base = 4 * t
                nc.vector.tensor_copy(
                    vaug[:, 0 * NKT + base:0 * NKT + base + 4, 0:64],
                    pt[:, 0:4, 0, 0:64])
                nc.vector.tensor_copy(
                    vaug[:, 1 * NKT + base:1 * NKT + base + 4, 0:64],
                    pt[:, 0:4, 0, 64:128])
                nc.vector.tensor_copy(
                    vaug[:, 2 * NKT + base:2 * NKT + base + 4, 0:64],
                    pt[:, 0:4, 1, 0:64])

            def qlo_of_t(t, kt):
                dm = kt - 4 * t
                return 128 * dm if dm >= 0 else 0

            def emit_S_for(h, t, p, tag="pT"):
                qt, kt_t, qb = qk_of[h]
                qlo_g = t * QC
                ps = psA.tile([128, 2 * QC], F32, tag="ps", name="ps")
                pT = sbp.tile([128, 2 * QC], BF16, tag=tag, name=tag)
                for half in range(2):
                    kt = 2 * p + half
                    qlo = qlo_of_t(t, kt)
                    nc.tensor.matmul(
                        ps[:, half * QC + qlo:(half + 1) * QC],
                        lhsT=kt_t[qb:qb + 64, kt * KT:(kt + 1) * KT],
                        rhs=qt[qb:qb + 64, qlo_g + qlo:qlo_g + QC],
                        start=True, stop=True,
                    )
                lo = qlo_of_t(t, 2 * p)
                nc.scalar.activation(
                    pT[:, lo:2 * QC], ps[:, lo:2 * QC], AF.Exp,
                    scale=SCALE,
                )
                for half in range(2):
                    kt = 2 * p + half
                    if kt - 4 * t >= 0:
                        o = half * QC + qlo_of_t(t, kt)
                        nc.vector.tensor_mul(
                            pT[:, o:o + 128], pT[:, o:o + 128],
                            trimask[:, :],
                        )
                return pT

            def emit_attn(h, t, pop_filler, pre=None, last=False):
                qt, kt_t, qb = qk_of[h]
                ydest, yrow = (yA, 0) if h == 0 else (yA, 64) if h == 1 else (yB, 0)
                qlo_g = t * QC
                py = psY.tile([128, QC], F32, tag="py", name="py")
                n_k = 4 * (t + 1)
                n_pair = n_k // 2

                def qlo_of(kt):
                    return qlo_of_t(t, kt)

                def emit_S(p):
                    if pre is not None and (h, p) in pre:
                        return pre.pop((h, p))
                    return emit_S_for(h, t, p)

                def emit_PV(p, pT):
                    for half in range(2):
                        kt = 2 * p + half
                        qlo = qlo_of(kt)
                        nc.tensor.matmul(
                            py[0:65, qlo:QC],
                            lhsT=vaug[:, h * NKT + kt, :],
                            rhs=pT[:, half * QC + qlo:(half + 1) * QC],
                            start=(kt == 0), stop=(kt == n_k - 1),
                        )

                pTs = {0: emit_S(0)}
                for p in range(n_pair):
                    if p + 1 < n_pair:
                        pTs[p + 1] = emit_S(p + 1)
                    pop_filler()
                    emit_PV(p, pTs.pop(p))
                    yield

                # softmax normalize: copy the denom row off PSUM (also
                # shifts partition 64 -> 0), reciprocal, then broadcast.
                den = sbp.tile([1, QC], F32, tag="den", name="den")
                nc.vector.tensor_copy(den[0:1, :], py[64:65, :])
                rec = sbp.tile([1, QC], F32, tag="rec", name="rec")
                nc.vector.reciprocal_approx_fast(rec[0:1, :], den[0:1, :])
                bcast = sbp.tile([64, QC], F32, tag="bcast", name="bcast")
                nc.gpsimd.partition_broadcast(bcast[:, :], rec[0:1, :])
                nc.vector.tensor_mul(
                    ydest[yrow:yrow + 64, qlo_g:qlo_g + QC],
                    py[0:64, :], bcast[:, :],
                )

            def emit_proj(ct, t, osb, last):
                ps = psA.tile([128, 2 * QC], F32, tag="ps", name="ps")
                nc.tensor.matmul(
                    ps[:, 0:QC],
                    lhsT=wproj_sb0[:, ct * 128:(ct + 1) * 128],
                    rhs=yA[:, t * QC:(t + 1) * QC],
                    start=True, stop=False,
                )
                nc.tensor.matmul(
                    ps[:, 0:QC],
                    lhsT=wproj_sb1[:, ct * 128:(ct + 1) * 128],
                    rhs=yB[:, t * QC:(t + 1) * QC],
                    start=False, stop=True,
                )
                if last and ct % 2 == 0:
                    # split tail psum->sbuf copies across ACT and DVE so the
                    # copy stream keeps up with the projection matmuls
                    nc.scalar.activation(osb[:, ct, :], ps[:, 0:QC], AF.Copy)
                else:
                    nc.vector.tensor_copy(osb[:, ct, :], ps[:, 0:QC])
                if last:
                    # tail: ship each 128-row block as soon as it's ready
                    nc.sync.dma_start(
                        yT_r[:, ct, t * QC:(t + 1) * QC], osb[:, ct, :])

            # ---- schedule: wavefront over q-chunks ----
            for m in range(NM):
                emit_qkv(m, 0)
            emit_vtrans(0)

            pre_pT = {}
            for t in range(NQC):
                fillers = []
                if t + 1 < NQC:
                    for m in range(NM):
                        fillers.append(lambda m=m: emit_qkv(m, t + 1))
                    fillers.append(lambda: emit_vtrans(t + 1))
                if t - 1 >= 0:
                    osb = sbo.tile([128, NCH, QC], BF16, tag="osb", name="osb")
                    for ct in range(NCH):
                        fillers.append(
                            lambda ct=ct, osb=osb: emit_proj(ct, t - 1, osb, False))
                    fillers.append(lambda tt=t - 1, osb=osb: nc.sync.dma_start(
                        yT_r[:, :, tt * QC:(tt + 1) * QC], osb[:, :, :]))


                # spread fillers evenly across this chunk's pair slots
                n_slots = HPC * 2 * (t + 1)
                fi = [0]

                def pop_filler(slot=[0], fi=fi, fillers=fillers, n_slots=n_slots):
                    slot[0] += 1
                    want = (slot[0] * len(fillers)) // n_slots
                    while fi[0] < want:
                        fillers[fi[0]]()
                        fi[0] += 1

                for h in range(HPC):
                    for _ in emit_attn(h, t, pop_filler,
                                       pre=pre_pT if t == NQC - 1 else None,
                                       last=(t == NQC - 1 and h == HPC - 1)):
                        pass
                while fi[0] < len(fillers):
                    fillers[fi[0]]()
                    fi[0] += 1

            osb = sbo.tile([128, NCH, QC], BF16, tag="osb", name="osb")
            for ct in range(NCH):
                emit_proj(ct, NQC - 1, osb, True)

    nc.finalize()
    return nc


def kernel(x, W_attn, b_attn, W_proj, b_proj):
    global LAST_RESULTS
    B = x.shape[0]
    x = np.asarray(x, np.float32)
    W_attn = np.asarray(W_attn, np.float32)
    b_attn = np.asarray(b_attn, np.float32)
    W_proj = np.asarray(W_proj, np.float32)
    b_proj = np.asarray(b_proj, np.float32)

    if "nc" not in _CACHE:
        _CACHE["nc"] = build()
    nc = _CACHE["nc"]

    in_maps = []
    for c in range(8):
        b, g = divmod(c, 4)
        heads = [3 * g + i for i in range(HPC)]
        h0, h1, h2 = heads
        Q = lambda h: W_attn[:, 64 * h:64 * h + 64]
        K = lambda h: W_attn[:, C + 64 * h:C + 64 * h + 64]
        V = lambda h: W_attn[:, 2 * C + 64 * h:2 * C + 64 * h + 64]
        bQ = lambda h: b_attn[64 * h:64 * h + 64]
        bK = lambda h: b_attn[C + 64 * h:C + 64 * h + 64]
        bV = lambda h: b_attn[2 * C + 64 * h:2 * C + 64 * h + 64]
        # m-tiles: [V0|V1], [V2|Q2], [Q0|Q1], [K0|K1], [0|K2]
        z64 = np.zeros((C, 64), np.float32)
        wqkvb = np.ascontiguousarray(np.concatenate(
            [V(h0), V(h1), V(h2), Q(h2), Q(h0), Q(h1), K(h0), K(h1),
             z64, K(h2)], 1)).astype(ml_dtypes.bfloat16)
        bcols = [bV(h0), bV(h1), bV(h2), bQ(h2), bQ(h0), bQ(h1),
                 bK(h0), bK(h1), np.zeros(64, np.float32), bK(h2)]
        bvec = np.concatenate(bcols)                     # [640] = 5 x 128
        battn = np.ascontiguousarray(bvec.reshape(NM, 128).T)  # [128, 5]
        wproj = np.concatenate(
            [W_proj[64 * h:64 * h + 64, :] for h in heads], 0)  # [192, C]
        xt = np.ascontiguousarray(x[b].T)
        in_maps.append({
            "xTb": xt.astype(ml_dtypes.bfloat16),
            "wqkvb": wqkvb,
            "battn": battn,
            "wproj": np.ascontiguousarray(wproj).astype(ml_dtypes.bfloat16),
        })

    res = run_bass_kernel_spmd(nc, in_maps, core_ids=list(range(8)))
    LAST_RESULTS = res

    out = np.zeros((B, T, C), np.float32)
    for c in range(8):
        b = c // 4
        out[b] += res.results[c]["yTb"].astype(np.float32).T
    out += b_proj
    return out
